# revision 31
# baseline (speedup 1.0000x reference)
"""Causal self-attention (B=2, N=2048, E=1024, H=16, HD=64) on 8 trn2 NeuronCores.

Sharding: (batch, head-group) — core c handles batch c//4 and heads
4*(c%4) .. 4*(c%4)+3.  Each core computes its heads' QKV projections,
causal attention, and a partial out-projection over its 256 feature rows
of Wout; the host sums the 4 partials per batch and adds all biases that
are affine in the output (bout and the v-bias term, which is constant
because softmax rows sum to 1).

v2 design (trace-driven):
  - ALL matmul operands are bf16 (fp32 psum accumulate).  fp32 K=64
    row-tiled score matmuls ran at half rate; bf16 restores 1 col/cycle
    and enables FWL weight loads.  Input DMA bytes halve.
  - attention works in q-chunks of 256 over 8 passes; per (k-tile,
    q-chunk) step ALL FOUR heads' S^T scores live in ONE [128,1024] psum
    tile (2 banks) -> ONE exp ACT per step over a [128,4,w] segmented AP
    (saves 352 fixed cycles/instr vs two) and exact windows.
  - causal mask on the PE: diagonal 128-blocks get an accumulating
    eye.T @ tmask matmul adding -1e9 below the diagonal before exp.
  - PV psum: one [65,1024] tile per pass (4 head slices sharing banks
    with per-bank single start=True + per-element has_written
    accumulation), pool bufs=2 -> passes double-buffer, so the
    normalization of pass c overlaps pass c+1 instead of stalling PE
    (the old 7-13us pass-boundary stalls re-throttled HAM).
  - normalization deferred: rcp(row 64) -> gpsimd partition_broadcast
    -> per-head mul into bf16 saT, emitted as next-pass fillers.
  - projections/out-projection stream as paced PE filler inside the
    passes; out-projection of chunk c runs in pass c+2, output DMA'd
    as bf16 (host sums partials in fp32).
"""

import numpy as np

import concourse.bass as bass
import concourse.tile as tile
from concourse import bacc, mybir
from concourse import bass_utils

B, N, E, H = 2, 2048, 1024, 16
HD = 64
NCORES = 8
NE = E // 128      # 8 e-chunks
NK = N // 128      # 16 k-tiles
NQ = N // 512      # 4 xt chunks of 512
QC = 256           # attention q-chunk
NP = N // QC       # 8 attention passes
F32 = mybir.dt.float32
BF16 = mybir.dt.bfloat16

_CACHE = {}


def _build_body(nc, tc, pools, dram, rep, stage=None):
    xt_d, wqk_d, wv_d, wout_d, bqk_d, eye_d, tmask_d, ones_d, out_d = dram
    (pconst, pqk, pvext, psa, pesb, psmall, pbc, pout, psps, ppv) = pools
    Exp = mybir.ActivationFunctionType.Exp

    # ---- input DMA: two HWDGE rings, big transfers, first-use order ------
    eye_sb = pconst.tile([128, 128], BF16, tag="eye", name="eye")
    nc.scalar.dma_start(eye_sb[:], eye_d[:])
    tmask_sb = pconst.tile([128, 128], BF16, tag="tmask", name="tmask")
    nc.scalar.dma_start(tmask_sb[:], tmask_d[:])
    bias_sb = {}
    for p in range(2):
        for i, nm in enumerate(("bq", "bk")):
            t = pconst.tile([128, 1], F32, tag=f"{nm}{p}", name=f"{nm}{p}")
            nc.scalar.dma_start(t[:], bqk_d[p, i])
            bias_sb[(p, nm)] = t
    wqk_sb = pconst.tile([128, NE * 512], BF16, tag="wqk", name="wqk")
    nc.scalar.dma_start(wqk_sb[:], wqk_d[:])
    # zero column for the K=128 group-closing matmuls (NOT memset: a
    # memset-written tile as PE moving operand crashes the HW)
    zk = pconst.tile([128, 1], BF16, tag="zk", name="zk")
    nc.vector.tensor_scalar_mul(zk[:], eye_sb[:, 0:1], 0.0)
    ones_sb = pconst.tile([128, 4], BF16, tag="ones", name="ones")
    nc.scalar.dma_start(ones_sb[:], ones_d[:])
    xt = []
    for nq in range(NQ):
        t = pconst.tile([128, NE * 512], BF16, tag=f"xt{nq}", name=f"xt{nq}")
        nc.sync.dma_start(t[:], xt_d[nq])
        xt.append(t)
    wv_sb = pconst.tile([128, NE * 256], BF16, tag="wv", name="wv")
    nc.scalar.dma_start(wv_sb[:], wv_d[:])
    wout_sb = pconst.tile([128, 2 * E], BF16, tag="wout", name="wout")
    nc.scalar.dma_start(wout_sb[:], wout_d[:])

    # ---- PE warmup while the DMAs stream: release the HAM clock gate -----
    wps = psps.tile([128, 128], F32, tag="sps", name="warmps")
    for i in range(40):
        nc.tensor.matmul(wps[:], eye_sb[:], eye_sb[:],
                         start=(i == 0), stop=(i == 39))
    wsb = psmall.tile([128, 128], F32, tag="warm", name="warmsb")
    nc.vector.tensor_copy(wsb[:], wps[:])

    # ---- persistent SBUF tensors ----------------------------------------
    qT, kT, saT = {}, {}, {}
    for p in range(2):
        qT[p] = pqk.tile([128, N], BF16, tag=f"qT{p}", name=f"qT{p}")
        kT[p] = pqk.tile([128, N], BF16, tag=f"kT{p}", name=f"kT{p}")
        saT[p] = psa.tile([128, N], BF16, tag=f"saT{p}", name=f"saT{p}")
    v_ext = []
    for nk in range(NK):
        vt = pvext.tile([128, 4 * 65], BF16, tag=f"vext{nk}", name=f"vext{nk}")
        # ones columns via DVE copy from a DMA'd tile (bf16 memset writes
        # bad values on HW)
        nc.vector.tensor_copy(
            vt[:].rearrange("p (h d) -> p h d", h=4)[:, :, 64:65],
            ones_sb[:].rearrange("p (h d) -> p h d", d=1))
        v_ext.append(vt)

    def emit_qkproj_group(p, which, nq):
        off = 0 if which == "k" else 256
        bias = bias_sb[(p, "bq" if which == "q" else "bk")]
        dst = qT[p] if which == "q" else kT[p]
        ps = psps.tile([128, 512], F32, tag="sps", name="projps")
        for e in range(NE):
            nc.tensor.matmul(
                ps[:], wqk_sb[:, e * 512 + off + p * 128:e * 512 + off + p * 128 + 128],
                xt[nq][:, e * 512:(e + 1) * 512],
                start=(e == 0), stop=(e == NE - 1))
        nc.vector.tensor_scalar_add(
            dst[:, nq * 512:(nq + 1) * 512], ps[:], bias[:])

    def emit_vproj_group(nk):
        if nk >= NK:
            return
        ps = psps.tile([128, 256], F32, tag="sps", name="vps")
        for e in range(NE):
            nc.tensor.matmul(
                ps[:], xt[nk // 4][:, e * 512 + (nk % 4) * 128:e * 512 + (nk % 4) * 128 + 128],
                wv_sb[:, e * 256:(e + 1) * 256],
                start=(e == 0), stop=(e == NE - 1))
        nc.vector.tensor_copy(
            v_ext[nk][:].rearrange("p (h d) -> p h d", h=4)[:, :, 0:64],
            ps[:].rearrange("p (h d) -> p h d", h=4))

    def emit_outproj(nk):
        ot = pout.tile([128, E], BF16, tag="outsb", name="outsb")
        for oc in range(2):
            ps = psps.tile([128, 512], F32, tag="sps", name="ops")
            for p in range(2):
                nc.tensor.matmul(
                    ps[:], saT[p][:, nk * 128:(nk + 1) * 128],
                    wout_sb[:, p * E + oc * 512:p * E + oc * 512 + 512],
                    start=(p == 0), stop=(p == 1))
            nc.vector.tensor_copy(ot[:, oc * 512:(oc + 1) * 512], ps[:])
        nc.sync.dma_start(out_d[nk], ot[:])

    def emit_norm_chain(qi, pv):
        """Normalize pass qi's pv accumulator into saT (as 3 closures)."""
        st = {}

        def c_rcp():
            # copy psum row 64 to SBUF first: reciprocal_approx_fast with
            # a PSUM source + partition rebase reads wrong data on HW
            dcp = psmall.tile([1, 1024], F32, tag="dcp", name=f"dcp{qi}")
            nc.vector.tensor_copy(dcp[:], pv[64:65, :])
            den = psmall.tile([1, 1024], F32, tag="den", name=f"den{qi}")
            nc.vector.reciprocal_approx_fast(den[:], dcp[:])
            st["den"] = den

        def c_bcast():
            bc = pbc.tile([64, 1024], F32, tag="bc", name=f"bc{qi}")
            nc.gpsimd.partition_broadcast(bc[:], st["den"][:])
            st["bc"] = bc

        def c_muls():
            for p in range(2):
                for hh in range(2):
                    off = (2 * p + hh) * 256
                    nc.vector.tensor_mul(
                        saT[p][hh * 64:hh * 64 + 64, qi * QC:(qi + 1) * QC],
                        pv[0:64, off:off + 256],
                        st["bc"][:, off:off + 256])
        return [c_rcp, c_bcast, c_muls]

    def attn_pass(qi, fillers):
        """Attention for q-chunk qi (QC=256 wide), all four heads.
        Steps kj=0..2qi+1; all 4 heads' S^T score tiles share one
        [128,1024] psum tile; ONE exp ACT per step over [128,4,w]
        segments; PV accumulates into one [65,1024] psum tile (pool
        bufs=2 so the next pass double-buffers); PV lags one step.
        Returns the pv tile for deferred normalization."""
        nsteps = 2 * qi + 2
        pv = ppv.tile([65, 1024], F32, tag="pv", name=f"pv{qi}")
        hi = qi * QC + QC
        pending = []
        nfill = 0

        def emit_pv(kj, lo, esbs):
            col0 = lo - qi * QC
            for p in range(2):
                for hh in range(2):
                    hloc = 2 * p + hh
                    nc.tensor.matmul(
                        pv[:, hloc * 256 + col0:hloc * 256 + 256],
                        v_ext[kj][:, hloc * 65:hloc * 65 + 65],
                        esbs[:, hloc * 256 + col0:hloc * 256 + hi - lo + col0],
                        start=(kj == 0 and hh == 0),
                        stop=(kj == 2 * qi + 1 and hh == 1))

        for kj in range(nsteps):
            q0 = 128 * kj
            lo = max(qi * QC, q0)
            w = hi - lo
            col0 = lo - qi * QC
            diag = (lo == q0)
            # Each (p,hh) score group: K=64 start=True MM, closed by a
            # K=128 stop=True MM (the diag mask, or an N=1 zero-add) --
            # a shared-bank group ENDING in a K=64 MM crashes the HW.
            # The closer follows ITS score before the next head's
            # start=True clears the bank's has_written bits.
            sps = psps.tile([128, 1024], F32, tag="sps", name="sps")
            for p in range(2):
                for hh in range(2):
                    rb = hh * 64
                    off = (2 * p + hh) * 256
                    nc.tensor.matmul(
                        sps[:, off + col0:off + col0 + w],
                        kT[p][rb:rb + 64, q0:q0 + 128],
                        qT[p][rb:rb + 64, lo:hi],
                        start=True, stop=False)
                    if diag:
                        nc.tensor.matmul(
                            sps[:, off + col0:off + col0 + 128],
                            eye_sb[:], tmask_sb[:],
                            start=False, stop=True)
                    else:
                        nc.tensor.matmul(
                            sps[:, off + col0:off + col0 + 1],
                            eye_sb[:], zk[:],
                            start=False, stop=True)
            esb = pesb.tile([128, 1024], BF16, tag="esb", name="esb")
            src = sps[:].rearrange("p (h c) -> p h c", h=4)[:, :, col0:col0 + w]
            dst = esb[:].rearrange("p (h c) -> p h c", h=4)[:, :, col0:col0 + w]
            nc.scalar.activation(dst, src, Exp)
            target = ((kj + 1) * len(fillers)) // nsteps
            while nfill < target:
                fillers[nfill]()
                nfill += 1
            pending.append((kj, lo, esb))
            if len(pending) > 1:
                emit_pv(*pending.pop(0))
        while pending:
            emit_pv(*pending.pop(0))
        while nfill < len(fillers):
            fillers[nfill]()
            nfill += 1
        return pv

    # ---- schedule -------------------------------------------------------
    # Prefix: only what pass 0 needs (qk chunk 0, v_ext[0..1]); the rest
    # streams in as paced PE filler.  Pass qi fillers: norm of pass qi-1,
    # v_ext for pass qi+1, qk chunk for passes 2nq (emitted at qi=2nq-1),
    # out-projection of chunk qi-2 (its norm ran during pass qi-1).
    for p in range(2):
        emit_qkproj_group(p, "k", 0)
        emit_qkproj_group(p, "q", 0)
    for nk in range(2):
        emit_vproj_group(nk)

    if stage in (36, 37, 38, 39, 41, 42, 43):
        # K=64 score groups closed by K=128 zero-accumulate MMs
        zw = 1 if stage in (36, 43) else 128
        zc = pconst.tile([128, 128], BF16, tag="zc", name="zc")
        if stage in (42, 43):
            nc.vector.tensor_scalar_mul(zc[:], tmask_sb[:], 0.0)
        else:
            nc.vector.memset(zc[:], 0.0)
        if stage == 38:
            zc = tmask_sb
        for p in range(2):
            nc.vector.memset(saT[p][:], 0.0)
        sps = psps.tile([128, 1024], F32, tag="sps", name="sps")
        for p in range(2):
            for hh in range(2):
                rb = hh * 64
                off = (2 * p + hh) * 256
                nc.tensor.matmul(
                    sps[:, off:off + 256],
                    kT[p][rb:rb + 64, 0:128], qT[p][rb:rb + 64, 0:256],
                    start=True, stop=(stage in (39, 41)))
                if stage not in (39, 41):
                    nc.tensor.matmul(
                        sps[:, off:off + zw], eye_sb[:], zc[:, 0:zw],
                        start=False, stop=True)
        esb = pesb.tile([128, 1024], BF16, tag="esb", name="esb")
        nc.scalar.activation(esb[:], sps[:], Exp)
        if stage != 41:
            ot = pout.tile([128, E], BF16, tag="outsb", name="dump")
            nc.vector.tensor_copy(ot[:, 0:1024], esb[:])
            nc.sync.dma_start(out_d[1], ot[:])
        else:
            # keep esb consumed but out of the DMA'd outputs
            dummy = pout.tile([128, E], BF16, tag="outsb", name="dump")
            nc.vector.tensor_copy(dummy[:, 0:1024], esb[:])
        emit_outproj(0)
        return
    if stage in (32, 33, 34, 35):
        # single attention pass qi=0 (2 steps), axis-isolated variants
        for p in range(2):
            nc.vector.memset(saT[p][:], 0.0)
        qi = 0
        pv = ppv.tile([65, 1024], F32, tag="pv", name="pv0")
        esbs_keep = []
        for kj in range(2):
            q0 = 128 * kj
            lo = q0
            w = 256 - 128 * kj
            col0 = lo
            sps = psps.tile([128, 1024], F32, tag="sps", name="sps")
            for p in range(2):
                for hh in range(2):
                    rb = hh * 64
                    off = (2 * p + hh) * 256
                    if stage == 35:
                        nc.tensor.matmul(
                            sps[:, off + col0:off + col0 + w],
                            kT[p][:, q0:q0 + 128], qT[p][:, lo:256],
                            start=True, stop=(stage == 34))
                    else:
                        nc.tensor.matmul(
                            sps[:, off + col0:off + col0 + w],
                            kT[p][rb:rb + 64, q0:q0 + 128],
                            qT[p][rb:rb + 64, lo:256],
                            start=True, stop=(stage == 34))
                    if stage != 34:
                        nc.tensor.matmul(
                            sps[:, off + col0:off + col0 + 128],
                            eye_sb[:], tmask_sb[:],
                            start=False, stop=True)
            esb = pesb.tile([128, 1024], BF16, tag="esb", name="esb")
            src = sps[:].rearrange("p (h c) -> p h c", h=4)[:, :, col0:col0 + w]
            dst = esb[:].rearrange("p (h c) -> p h c", h=4)[:, :, col0:col0 + w]
            nc.scalar.activation(dst, src, Exp)
            esbs_keep.append((kj, lo, w, col0, esb))
        if stage != 33:
            for kj, lo, w, col0, esb in esbs_keep:
                for p in range(2):
                    for hh in range(2):
                        hloc = 2 * p + hh
                        nc.tensor.matmul(
                            pv[:, hloc * 256 + col0:hloc * 256 + 256],
                            v_ext[kj][:, hloc * 65:hloc * 65 + 65],
                            esb[:, hloc * 256 + col0:hloc * 256 + w + col0],
                            start=(kj == 0 and hh == 0),
                            stop=(kj == 1 and hh == 1))
            for fn in emit_norm_chain(0, pv):
                fn()
        else:
            ot = pout.tile([128, E], BF16, tag="outsb", name="dump")
            for kj, lo, w, col0, esb in esbs_keep:
                nc.vector.tensor_copy(ot[:, 0:1024], esb[:])
            nc.sync.dma_start(out_d[1], ot[:])
        emit_outproj(0)
        return
    if stage in (1, 2, 3, 30, 31):
        for p in range(2):
            nc.vector.memset(saT[p][:], 0.0)
        if stage >= 2:
            for nq in range(1, NQ):
                for p in range(2):
                    emit_qkproj_group(p, "k", nq)
                    emit_qkproj_group(p, "q", nq)
            for nk in range(2, NK):
                emit_vproj_group(nk)
        if stage == 3:
            for qi in range(NP):
                attn_pass(qi, [])
        elif stage == 30:
            # attention core only, 2 passes
            for qi in range(2):
                attn_pass(qi, [])
        elif stage == 31:
            # attention with inline norms, no fillers
            for qi in range(NP):
                pv = attn_pass(qi, [])
                for fn in emit_norm_chain(qi, pv):
                    fn()
        for nk in range(NK):
            emit_outproj(nk)
        return
    if stage == 4:
        for nq in range(1, NQ):
            for p in range(2):
                emit_qkproj_group(p, "k", nq)
                emit_qkproj_group(p, "q", nq)
        for nk in range(2, NK):
            emit_vproj_group(nk)
        pv_prev = None
        for qi in range(NP):
            fillers = []
            if pv_prev is not None:
                fillers += emit_norm_chain(qi - 1, pv_prev)
            pv_prev = attn_pass(qi, fillers)
        for fn in emit_norm_chain(NP - 1, pv_prev):
            fn()
        for nk in range(NK):
            emit_outproj(nk)
        return

    if stage == 5:
        # v3 structure but norm runs inline at pass end (not deferred)
        for qi in range(NP):
            fillers = []
            fillers.append(lambda nk=2 * qi + 2: emit_vproj_group(nk))
            fillers.append(lambda nk=2 * qi + 3: emit_vproj_group(nk))
            if qi % 2 == 1 and (qi + 1) // 2 <= 3:
                nq = (qi + 1) // 2
                for p in range(2):
                    fillers.append(
                        lambda p=p, nq=nq: emit_qkproj_group(p, "k", nq))
                    fillers.append(
                        lambda p=p, nq=nq: emit_qkproj_group(p, "q", nq))
            if qi >= 2:
                fillers.append(lambda nk=2 * (qi - 2): emit_outproj(nk))
                fillers.append(lambda nk=2 * (qi - 2) + 1: emit_outproj(nk))
            pv = attn_pass(qi, fillers)
            for fn in emit_norm_chain(qi, pv):
                fn()
        for nk in range(12, NK):
            emit_outproj(nk)
        return

    pv_prev = None
    for qi in range(NP):
        fillers = []
        if pv_prev is not None:
            fillers += emit_norm_chain(qi - 1, pv_prev)
        fillers.append(lambda nk=2 * qi + 2: emit_vproj_group(nk))
        fillers.append(lambda nk=2 * qi + 3: emit_vproj_group(nk))
        if qi % 2 == 1 and (qi + 1) // 2 <= 3:
            nq = (qi + 1) // 2
            for p in range(2):
                fillers.append(
                    lambda p=p, nq=nq: emit_qkproj_group(p, "k", nq))
                fillers.append(
                    lambda p=p, nq=nq: emit_qkproj_group(p, "q", nq))
        if qi >= 2:
            fillers.append(lambda nk=2 * (qi - 2): emit_outproj(nk))
            fillers.append(lambda nk=2 * (qi - 2) + 1: emit_outproj(nk))
        pv_prev = attn_pass(qi, fillers)

    emit_outproj(12)
    emit_outproj(13)
    for fn in emit_norm_chain(NP - 1, pv_prev):
        fn()
    emit_outproj(14)
    emit_outproj(15)


def build_nc(reps=1, loop=None, stage=None):
    nc = bacc.Bacc("TRN2", target_bir_lowering=False, debug=False,
                   enable_asserts=True, num_devices=NCORES)
    xt_d = nc.dram_tensor("xt", [NQ, 128, NE * 512], BF16,
                          kind="ExternalInput").ap()
    wqk_d = nc.dram_tensor("wqk", [128, NE * 512], BF16,
                           kind="ExternalInput").ap()
    wv_d = nc.dram_tensor("wv", [128, NE * 256], BF16,
                          kind="ExternalInput").ap()
    wout_d = nc.dram_tensor("wout", [128, 2 * E], BF16,
                            kind="ExternalInput").ap()
    bqk_d = nc.dram_tensor("bqk", [2, 2, 128, 1], F32,
                           kind="ExternalInput").ap()
    eye_d = nc.dram_tensor("eye", [128, 128], BF16, kind="ExternalInput").ap()
    tmask_d = nc.dram_tensor("tmask", [128, 128], BF16,
                             kind="ExternalInput").ap()
    ones_d = nc.dram_tensor("ones", [128, 4], BF16,
                            kind="ExternalInput").ap()
    out_d = nc.dram_tensor("out", [NK, 128, E], BF16,
                           kind="ExternalOutput").ap()
    dram = (xt_d, wqk_d, wv_d, wout_d, bqk_d, eye_d, tmask_d, ones_d, out_d)

    with tile.TileContext(nc) as tc:
        from contextlib import ExitStack
        with ExitStack() as ctx:
            pconst = ctx.enter_context(tc.tile_pool(name="const", bufs=1))
            pqk = ctx.enter_context(tc.tile_pool(name="qk", bufs=1))
            pvext = ctx.enter_context(tc.tile_pool(name="vext", bufs=1))
            psa = ctx.enter_context(tc.tile_pool(name="sa", bufs=1))
            pesb = ctx.enter_context(tc.tile_pool(name="esb", bufs=4))
            psmall = ctx.enter_context(tc.tile_pool(name="small", bufs=2))
            pbc = ctx.enter_context(tc.tile_pool(name="bc", bufs=2))
            pout = ctx.enter_context(tc.tile_pool(name="outsb", bufs=2))
            psps = ctx.enter_context(
                tc.tile_pool(name="sps", bufs=2, space="PSUM"))
            ppv = ctx.enter_context(
                tc.tile_pool(name="pv", bufs=2, space="PSUM"))
            pools = (pconst, pqk, pvext, psa, pesb, psmall, pbc, pout,
                     psps, ppv)
            if loop is not None:
                with tc.For_i(0, loop, 1,
                              hint_engines=(mybir.EngineType.PE,
                                            mybir.EngineType.Activation,
                                            mybir.EngineType.DVE,
                                            mybir.EngineType.SP)):
                    _build_body(nc, tc, pools, dram, 0)
            else:
                for r in range(reps):
                    _build_body(nc, tc, pools, dram, r, stage=stage)
    nc.compile()
    return nc


def make_in_maps(x, Wqkv, bqkv, Wout):
    """Per-core input dicts. Shapes per reference: x[B,N,E], Wqkv[H,E,3HD],
    bqkv[H,3HD], Wout[E,E].  Split: cols 0:64=k, 64:128=q, 128:192=v."""
    import ml_dtypes
    bf16 = ml_dtypes.bfloat16
    Wk = Wqkv[:, :, 0:HD]
    Wq = Wqkv[:, :, HD:2 * HD] * (1.0 / np.sqrt(HD))
    Wv = Wqkv[:, :, 2 * HD:3 * HD]
    bk = bqkv[:, 0:HD]
    bq = bqkv[:, HD:2 * HD] * (1.0 / np.sqrt(HD))

    eye = np.eye(128, dtype=np.float32)
    # S^T tile rows are k, cols are q: keep q >= k -> add -1e9 where q < k
    tmask = np.tril(np.full((128, 128), -1e9, dtype=np.float32), k=-1)

    in_maps = []
    for c in range(NCORES):
        b, hg = divmod(c, 4)
        hs = slice(4 * hg, 4 * hg + 4)

        # xt[nq, row, e*512 + col] = x[b, nq*512+col, e*128+row]
        xT = np.ascontiguousarray(x[b].T)          # [E, N]
        xt = (xT.reshape(NE, 128, NQ, 512)
                .transpose(2, 1, 0, 3)
                .reshape(NQ, 128, NE * 512))

        # wqk[row, e*512 + which*256 + p*128 + hh*64 + d]
        #   = W_which[4hg + 2p + hh, e*128 + row, d]
        wqk = np.empty((128, NE, 2, 2, 2, HD), dtype=np.float32)
        for which, W in ((0, Wk), (1, Wq)):
            w4 = np.asarray(W[hs]).reshape(2, 2, NE, 128, HD)
            wqk[:, :, which] = w4.transpose(3, 2, 0, 1, 4)
        wqk = wqk.reshape(128, NE * 512)

        # wv[row, e*256 + hloc*64 + d] = Wv[4hg + hloc, e*128 + row, d]
        wv = (np.asarray(Wv[hs]).reshape(4, NE, 128, HD)
                .transpose(2, 1, 0, 3).reshape(128, NE * 256))

        # wout[row, p*E + c] = Wout[4hg*HD + p*128 + row, c]
        wout = (np.asarray(Wout[4 * hg * HD:(4 * hg + 4) * HD])
                .reshape(2, 128, E).transpose(1, 0, 2).reshape(128, 2 * E))

        bqk = np.stack([
            np.stack([bq[4 * hg + 2 * p:4 * hg + 2 * p + 2].reshape(128),
                      bk[4 * hg + 2 * p:4 * hg + 2 * p + 2].reshape(128)])
            for p in range(2)]).reshape(2, 2, 128, 1)
        in_maps.append({
            "xt": np.ascontiguousarray(xt).astype(bf16),
            "wqk": np.ascontiguousarray(wqk).astype(bf16),
            "wv": np.ascontiguousarray(wv).astype(bf16),
            "wout": np.ascontiguousarray(wout).astype(bf16),
            "bqk": bqk.astype(np.float32),
            "eye": eye.astype(bf16),
            "tmask": tmask.astype(bf16),
            "ones": np.ones((128, 4), dtype=np.float32).astype(bf16),
        })
    return in_maps


def combine(results, bqkv, Wout, bout):
    bv = bqkv[:, 2 * HD:3 * HD].reshape(E)          # concat over heads
    const_row = bv @ Wout + bout                     # [E]
    out = np.zeros((B, N, E), dtype=np.float32)
    for c in range(NCORES):
        b = c // 4
        out[b] += results[c]["out"].reshape(N, E).astype(np.float32)
    out += const_row[None, None, :].astype(np.float32)
    return out


def kernel(x, Wqkv, bqkv, Wout, bout):
    x = np.asarray(x, dtype=np.float32)
    Wqkv = np.asarray(Wqkv, dtype=np.float32)
    bqkv = np.asarray(bqkv, dtype=np.float32)
    Wout = np.asarray(Wout, dtype=np.float32)
    bout = np.asarray(bout, dtype=np.float32)

    if "nc" not in _CACHE:
        _CACHE["nc"] = build_nc(reps=1)
    nc = _CACHE["nc"]
    in_maps = make_in_maps(x, Wqkv, bqkv, Wout)
    res = bass_utils.run_bass_kernel_spmd(
        nc, in_maps, core_ids=list(range(NCORES)), trace=False)
    return combine(res.results, bqkv, Wout, bout)


# revision 32
# speedup vs baseline: 1.2593x; 1.2593x over previous
"""Causal self-attention (B=2, N=2048, E=1024, H=16, HD=64) on 8 trn2 NeuronCores.

Sharding: (batch, head-group) — core c handles batch c//4 and heads
4*(c%4) .. 4*(c%4)+3.  Each core computes its heads' QKV projections,
causal attention, and a partial out-projection over its 256 feature rows
of Wout; the host sums the 4 partials per batch and adds all biases that
are affine in the output (bout and the v-bias term, which is constant
because softmax rows sum to 1).

v6 design (trace-driven):
  - ALL matmul operands are bf16 (fp32 psum accumulate): fp32 K=64
    row-tiled score matmuls ran at half rate; bf16 restores 1 col/cycle
    and FWL halves weight loads.  Input/output DMA bytes halve.
  - attention in q-chunks of 512 over 4 passes; per (k-tile, pair)
    sub-step both heads' S^T scores go to one [128,1024] psum tile with
    each head's 512-col block exactly one psum bank (no bank sharing:
    a shared-bank group ending in a K=64 matmul crashes the HW), pool
    bufs=2 so sub-steps double-buffer.  One exp ACT per sub-step over
    exact [128,2,w] segments.
  - causal mask on the PE: diagonal 128-blocks get an accumulating
    eye.T @ tmask matmul (K=128 group closer) before exp.
  - PV per pair accumulates into a [65,1024] psum tile (head blocks =
    own banks, K=128 groups); normalization is DEFERRED into the next
    pass as paced fillers: psum row 64 -> SBUF copy -> fast reciprocal
    (rcp straight from PSUM reads wrong data on HW) -> gpsimd
    partition broadcast -> per-head multiply into bf16 saT.
  - projections stream as paced PE filler; interior qk chunks (1,2) are
    produced two-at-a-time per stationary weight load (halves LDW);
    out-projection of chunk c runs in pass c+1 after c's norm; output
    DMA'd as bf16 (host sums partials in fp32).
  - HAM: QC=256 variant measured 172us clock-gated (small matmuls drop
    PE duty below the activity threshold) — keep matmuls >=256 cols.
"""

import numpy as np

import concourse.bass as bass
import concourse.tile as tile
from concourse import bacc, mybir
from concourse import bass_utils

B, N, E, H = 2, 2048, 1024, 16
HD = 64
NCORES = 8
NE = E // 128      # 8 e-chunks
NK = N // 128      # 16 k-tiles
NQ = N // 512      # 4 xt/attention chunks of 512
F32 = mybir.dt.float32
BF16 = mybir.dt.bfloat16

_CACHE = {}


def _build_body(nc, tc, pools, dram, rep):
    xt_d, wqk_d, wv_d, wout_d, bqk_d, eye_d, tmask_d, ones_d, out_d = dram
    (pconst, pqk, pvext, psa, pesb, psmall, pbc, pout, psps, ppv) = pools
    Exp = mybir.ActivationFunctionType.Exp

    # ---- input DMA: two HWDGE rings, big transfers, first-use order ------
    eye_sb = pconst.tile([128, 128], BF16, tag="eye", name="eye")
    nc.scalar.dma_start(eye_sb[:], eye_d[:])
    tmask_sb = pconst.tile([128, 128], BF16, tag="tmask", name="tmask")
    nc.scalar.dma_start(tmask_sb[:], tmask_d[:])
    ones_sb = pconst.tile([128, 4], BF16, tag="ones", name="ones")
    nc.scalar.dma_start(ones_sb[:], ones_d[:])
    bias_sb = {}
    for p in range(2):
        for i, nm in enumerate(("bq", "bk")):
            t = pconst.tile([128, 1], F32, tag=f"{nm}{p}", name=f"{nm}{p}")
            nc.scalar.dma_start(t[:], bqk_d[p, i])
            bias_sb[(p, nm)] = t
    wqk_sb = pconst.tile([128, NE * 512], BF16, tag="wqk", name="wqk")
    nc.scalar.dma_start(wqk_sb[:], wqk_d[:])
    xt = []
    for nq in range(NQ):
        t = pconst.tile([128, NE * 512], BF16, tag=f"xt{nq}", name=f"xt{nq}")
        nc.sync.dma_start(t[:], xt_d[nq])
        xt.append(t)
    wv_sb = pconst.tile([128, NE * 256], BF16, tag="wv", name="wv")
    nc.scalar.dma_start(wv_sb[:], wv_d[:])
    wout_sb = pconst.tile([128, 2 * E], BF16, tag="wout", name="wout")
    nc.scalar.dma_start(wout_sb[:], wout_d[:])

    # ---- PE warmup while the DMAs stream: release the HAM clock gate -----
    wps = psps.tile([128, 128], F32, tag="sps", name="warmps")
    for i in range(40):
        nc.tensor.matmul(wps[:], eye_sb[:], eye_sb[:],
                         start=(i == 0), stop=(i == 39))
    wsb = psmall.tile([128, 128], F32, tag="warm", name="warmsb")
    nc.vector.tensor_copy(wsb[:], wps[:])

    # ---- persistent SBUF tensors ----------------------------------------
    qT, kT, saT = {}, {}, {}
    for p in range(2):
        qT[p] = pqk.tile([128, N], BF16, tag=f"qT{p}", name=f"qT{p}")
        kT[p] = pqk.tile([128, N], BF16, tag=f"kT{p}", name=f"kT{p}")
        saT[p] = psa.tile([128, N], BF16, tag=f"saT{p}", name=f"saT{p}")
    v_ext = []
    for nk in range(NK):
        vt = pvext.tile([128, 4 * 65], BF16, tag=f"vext{nk}", name=f"vext{nk}")
        # ones columns via DVE copy from a DMA'd tile (memset tiles are
        # HW-hazardous as PE operands; DMA+copy is proven)
        nc.vector.tensor_copy(
            vt[:].rearrange("p (h d) -> p h d", h=4)[:, :, 64:65],
            ones_sb[:].rearrange("p (h d) -> p h d", d=1))
        v_ext.append(vt)

    def emit_qkproj(p, which, nqs):
        """Project q or k for pair p for one or two 512-col chunks,
        sharing each stationary weight load across the chunks."""
        off = 0 if which == "k" else 256
        bias = bias_sb[(p, "bq" if which == "q" else "bk")]
        dst = qT[p] if which == "q" else kT[p]
        ps = psps.tile([128, 512 * len(nqs)], F32, tag="sps", name="projps")
        for e in range(NE):
            for j, nq in enumerate(nqs):
                nc.tensor.matmul(
                    ps[:, j * 512:(j + 1) * 512],
                    wqk_sb[:, e * 512 + off + p * 128:e * 512 + off + p * 128 + 128],
                    xt[nq][:, e * 512:(e + 1) * 512],
                    start=(e == 0), stop=(e == NE - 1))
        for j, nq in enumerate(nqs):
            nc.vector.tensor_scalar_add(
                dst[:, nq * 512:(nq + 1) * 512],
                ps[:, j * 512:(j + 1) * 512], bias[:])

    def emit_vproj_group(nk):
        if nk >= NK:
            return
        ps = psps.tile([128, 256], F32, tag="sps", name="vps")
        for e in range(NE):
            nc.tensor.matmul(
                ps[:], xt[nk // 4][:, e * 512 + (nk % 4) * 128:e * 512 + (nk % 4) * 128 + 128],
                wv_sb[:, e * 256:(e + 1) * 256],
                start=(e == 0), stop=(e == NE - 1))
        nc.vector.tensor_copy(
            v_ext[nk][:].rearrange("p (h d) -> p h d", h=4)[:, :, 0:64],
            ps[:].rearrange("p (h d) -> p h d", h=4))

    def emit_outproj(nk):
        ot = pout.tile([128, E], BF16, tag="outsb", name="outsb")
        for oc in range(2):
            ps = psps.tile([128, 512], F32, tag="sps", name="ops")
            for p in range(2):
                nc.tensor.matmul(
                    ps[:], saT[p][:, nk * 128:(nk + 1) * 128],
                    wout_sb[:, p * E + oc * 512:p * E + oc * 512 + 512],
                    start=(p == 0), stop=(p == 1))
            nc.vector.tensor_copy(ot[:, oc * 512:(oc + 1) * 512], ps[:])
        nc.sync.dma_start(out_d[nk], ot[:])

    def emit_norm_chain(qi, pvs):
        """Normalize pass qi's pv accumulators into saT (closure list,
        emitted as paced fillers inside pass qi+1)."""
        st = {}
        out = []
        for p in range(2):
            def c_rcp(p=p):
                # copy psum row 64 to SBUF first: rcp_approx_fast with a
                # PSUM source + partition rebase reads wrong data on HW
                dcp = psmall.tile([1, 1024], F32, tag="dcp", name=f"dcp{qi}_{p}")
                nc.vector.tensor_copy(dcp[:], pvs[p][64:65, :])
                den = psmall.tile([1, 1024], F32, tag="den", name=f"den{qi}_{p}")
                nc.vector.reciprocal_approx_fast(den[:], dcp[:])
                st[("den", p)] = den

            def c_bcast(p=p):
                bc = pbc.tile([64, 1024], F32, tag="bc", name=f"bc{qi}_{p}")
                nc.gpsimd.partition_broadcast(bc[:], st[("den", p)][:])
                st[("bc", p)] = bc

            def c_muls(p=p):
                for hh in range(2):
                    nc.vector.tensor_mul(
                        saT[p][hh * 64:hh * 64 + 64, qi * 512:(qi + 1) * 512],
                        pvs[p][0:64, hh * 512:(hh + 1) * 512],
                        st[("bc", p)][:, hh * 512:(hh + 1) * 512])
            out += [c_rcp, c_bcast, c_muls]
        return out

    def attn_pass(qi, fillers):
        """Attention for q-chunk qi (512 wide), both pairs, four heads.
        Per (k-tile, pair) sub-step: two K=64 row-tiled score MMs (one
        per head, each into its OWN psum bank), causal mask accumulate
        on diagonal blocks, one exact segmented exp ACT; PV lags one
        k-tile.  Returns {p: pv tile} for deferred normalization."""
        nsteps = 4 * qi + 4
        pvs = {p: ppv.tile([65, 1024], F32, tag="pv", name=f"pv{qi}_{p}")
               for p in range(2)}
        hi = qi * 512 + 512
        pending = []
        nfill = 0
        nsub = nsteps * 2

        def emit_pv(kj, lo, p, esb):
            col0 = lo - qi * 512
            for hh in range(2):
                hloc = 2 * p + hh
                nc.tensor.matmul(
                    pvs[p][:, hh * 512 + col0:hh * 512 + 512],
                    v_ext[kj][:, hloc * 65:hloc * 65 + 65],
                    esb[:, hh * 512 + col0:hh * 512 + hi - lo + col0],
                    start=(kj == 0), stop=(kj == 4 * qi + 3))

        for kj in range(nsteps):
            q0 = 128 * kj
            lo = max(qi * 512, q0)
            w = hi - lo
            col0 = lo - qi * 512
            diag = (lo == q0)
            for p in range(2):
                sps = psps.tile([128, 1024], F32, tag="sps", name="sps")
                for hh in range(2):
                    rb = hh * 64
                    nc.tensor.matmul(
                        sps[:, hh * 512 + col0:hh * 512 + col0 + w],
                        kT[p][rb:rb + 64, q0:q0 + 128],
                        qT[p][rb:rb + 64, lo:hi],
                        start=True, stop=not diag)
                    if diag:
                        nc.tensor.matmul(
                            sps[:, hh * 512 + col0:hh * 512 + col0 + 128],
                            eye_sb[:], tmask_sb[:],
                            start=False, stop=True)
                esb = pesb.tile([128, 1024], BF16, tag="esb", name="esb")
                src = sps[:].rearrange("p (h c) -> p h c", h=2)[:, :, col0:col0 + w]
                dst = esb[:].rearrange("p (h c) -> p h c", h=2)[:, :, col0:col0 + w]
                nc.scalar.activation(dst, src, Exp)
                target = ((2 * kj + p + 1) * len(fillers)) // nsub
                while nfill < target:
                    fillers[nfill]()
                    nfill += 1
                pending.append((kj, lo, p, esb))
                if len(pending) > 2:
                    emit_pv(*pending.pop(0))
        while pending:
            emit_pv(*pending.pop(0))
        while nfill < len(fillers):
            fillers[nfill]()
            nfill += 1
        return pvs

    # ---- schedule -------------------------------------------------------
    # Prefix: qk chunk 0 and v_ext[0..3] (all pass 0 needs).  Pass qi
    # fillers: norm of pass qi-1, then projections for later passes
    # (chunks 1+2 produced two-at-a-time per weight load), then the
    # out-projection of chunk qi-1 (its norm ran earlier this pass).
    for p in range(2):
        emit_qkproj(p, "k", [0])
        emit_qkproj(p, "q", [0])
    for nk in range(4):
        emit_vproj_group(nk)

    pv_prev = None
    for qi in range(NQ):
        fillers = []
        if pv_prev is not None:
            fillers += emit_norm_chain(qi - 1, pv_prev)
        if qi == 0:
            for p in range(2):
                fillers.append(lambda p=p: emit_qkproj(p, "k", [1, 2]))
                fillers.append(lambda p=p: emit_qkproj(p, "q", [1, 2]))
        elif qi == 2:
            for p in range(2):
                fillers.append(lambda p=p: emit_qkproj(p, "k", [3]))
                fillers.append(lambda p=p: emit_qkproj(p, "q", [3]))
        for nk in range(4 * qi + 4, 4 * qi + 8):
            fillers.append(lambda nk=nk: emit_vproj_group(nk))
        if qi >= 1:
            for nk in range(4 * (qi - 1), 4 * qi):
                fillers.append(lambda nk=nk: emit_outproj(nk))
        pv_prev = attn_pass(qi, fillers)

    for fn in emit_norm_chain(NQ - 1, pv_prev):
        fn()
    for nk in range(12, NK):
        emit_outproj(nk)


def build_nc(reps=1, loop=None):
    nc = bacc.Bacc("TRN2", target_bir_lowering=False, debug=False,
                   enable_asserts=True, num_devices=NCORES)
    xt_d = nc.dram_tensor("xt", [NQ, 128, NE * 512], BF16,
                          kind="ExternalInput").ap()
    wqk_d = nc.dram_tensor("wqk", [128, NE * 512], BF16,
                           kind="ExternalInput").ap()
    wv_d = nc.dram_tensor("wv", [128, NE * 256], BF16,
                          kind="ExternalInput").ap()
    wout_d = nc.dram_tensor("wout", [128, 2 * E], BF16,
                            kind="ExternalInput").ap()
    bqk_d = nc.dram_tensor("bqk", [2, 2, 128, 1], F32,
                           kind="ExternalInput").ap()
    eye_d = nc.dram_tensor("eye", [128, 128], BF16, kind="ExternalInput").ap()
    tmask_d = nc.dram_tensor("tmask", [128, 128], BF16,
                             kind="ExternalInput").ap()
    ones_d = nc.dram_tensor("ones", [128, 4], BF16,
                            kind="ExternalInput").ap()
    out_d = nc.dram_tensor("out", [NK, 128, E], BF16,
                           kind="ExternalOutput").ap()
    dram = (xt_d, wqk_d, wv_d, wout_d, bqk_d, eye_d, tmask_d, ones_d, out_d)

    with tile.TileContext(nc) as tc:
        from contextlib import ExitStack
        with ExitStack() as ctx:
            pconst = ctx.enter_context(tc.tile_pool(name="const", bufs=1))
            pqk = ctx.enter_context(tc.tile_pool(name="qk", bufs=1))
            pvext = ctx.enter_context(tc.tile_pool(name="vext", bufs=1))
            psa = ctx.enter_context(tc.tile_pool(name="sa", bufs=1))
            pesb = ctx.enter_context(tc.tile_pool(name="esb", bufs=4))
            psmall = ctx.enter_context(tc.tile_pool(name="small", bufs=2))
            pbc = ctx.enter_context(tc.tile_pool(name="bc", bufs=2))
            pout = ctx.enter_context(tc.tile_pool(name="outsb", bufs=2))
            psps = ctx.enter_context(
                tc.tile_pool(name="sps", bufs=2, space="PSUM"))
            ppv = ctx.enter_context(
                tc.tile_pool(name="pv", bufs=2, space="PSUM"))
            pools = (pconst, pqk, pvext, psa, pesb, psmall, pbc, pout,
                     psps, ppv)
            if loop is not None:
                with tc.For_i(0, loop, 1,
                              hint_engines=(mybir.EngineType.PE,
                                            mybir.EngineType.Activation,
                                            mybir.EngineType.DVE,
                                            mybir.EngineType.SP)):
                    _build_body(nc, tc, pools, dram, 0)
            else:
                for r in range(reps):
                    _build_body(nc, tc, pools, dram, r)
    nc.compile()
    return nc


def make_in_maps(x, Wqkv, bqkv, Wout):
    """Per-core input dicts. Shapes per reference: x[B,N,E], Wqkv[H,E,3HD],
    bqkv[H,3HD], Wout[E,E].  Split: cols 0:64=k, 64:128=q, 128:192=v."""
    import ml_dtypes
    bf16 = ml_dtypes.bfloat16
    Wk = Wqkv[:, :, 0:HD]
    Wq = Wqkv[:, :, HD:2 * HD] * (1.0 / np.sqrt(HD))
    Wv = Wqkv[:, :, 2 * HD:3 * HD]
    bk = bqkv[:, 0:HD]
    bq = bqkv[:, HD:2 * HD] * (1.0 / np.sqrt(HD))

    eye = np.eye(128, dtype=np.float32)
    # S^T tile rows are k, cols are q: keep q >= k -> add -1e9 where q < k
    tmask = np.tril(np.full((128, 128), -1e9, dtype=np.float32), k=-1)

    in_maps = []
    for c in range(NCORES):
        b, hg = divmod(c, 4)
        hs = slice(4 * hg, 4 * hg + 4)

        # xt[nq, row, e*512 + col] = x[b, nq*512+col, e*128+row]
        xT = np.ascontiguousarray(x[b].T)          # [E, N]
        xt = (xT.reshape(NE, 128, NQ, 512)
                .transpose(2, 1, 0, 3)
                .reshape(NQ, 128, NE * 512))

        # wqk[row, e*512 + which*256 + p*128 + hh*64 + d]
        #   = W_which[4hg + 2p + hh, e*128 + row, d]
        wqk = np.empty((128, NE, 2, 2, 2, HD), dtype=np.float32)
        for which, W in ((0, Wk), (1, Wq)):
            w4 = np.asarray(W[hs]).reshape(2, 2, NE, 128, HD)
            wqk[:, :, which] = w4.transpose(3, 2, 0, 1, 4)
        wqk = wqk.reshape(128, NE * 512)

        # wv[row, e*256 + hloc*64 + d] = Wv[4hg + hloc, e*128 + row, d]
        wv = (np.asarray(Wv[hs]).reshape(4, NE, 128, HD)
                .transpose(2, 1, 0, 3).reshape(128, NE * 256))

        # wout[row, p*E + c] = Wout[4hg*HD + p*128 + row, c]
        wout = (np.asarray(Wout[4 * hg * HD:(4 * hg + 4) * HD])
                .reshape(2, 128, E).transpose(1, 0, 2).reshape(128, 2 * E))

        bqk = np.stack([
            np.stack([bq[4 * hg + 2 * p:4 * hg + 2 * p + 2].reshape(128),
                      bk[4 * hg + 2 * p:4 * hg + 2 * p + 2].reshape(128)])
            for p in range(2)]).reshape(2, 2, 128, 1)
        in_maps.append({
            "xt": np.ascontiguousarray(xt).astype(bf16),
            "wqk": np.ascontiguousarray(wqk).astype(bf16),
            "wv": np.ascontiguousarray(wv).astype(bf16),
            "wout": np.ascontiguousarray(wout).astype(bf16),
            "bqk": bqk.astype(np.float32),
            "eye": eye.astype(bf16),
            "tmask": tmask.astype(bf16),
            "ones": np.ones((128, 4), dtype=np.float32).astype(bf16),
        })
    return in_maps


def combine(results, bqkv, Wout, bout):
    bv = bqkv[:, 2 * HD:3 * HD].reshape(E)          # concat over heads
    const_row = bv @ Wout + bout                     # [E]
    out = np.zeros((B, N, E), dtype=np.float32)
    for c in range(NCORES):
        b = c // 4
        out[b] += results[c]["out"].reshape(N, E).astype(np.float32)
    out += const_row[None, None, :].astype(np.float32)
    return out


def kernel(x, Wqkv, bqkv, Wout, bout):
    x = np.asarray(x, dtype=np.float32)
    Wqkv = np.asarray(Wqkv, dtype=np.float32)
    bqkv = np.asarray(bqkv, dtype=np.float32)
    Wout = np.asarray(Wout, dtype=np.float32)
    bout = np.asarray(bout, dtype=np.float32)

    if "nc" not in _CACHE:
        _CACHE["nc"] = build_nc(reps=1)
    nc = _CACHE["nc"]
    in_maps = make_in_maps(x, Wqkv, bqkv, Wout)
    res = bass_utils.run_bass_kernel_spmd(
        nc, in_maps, core_ids=list(range(NCORES)), trace=False)
    return combine(res.results, bqkv, Wout, bout)


# revision 35
# speedup vs baseline: 1.3265x; 1.0534x over previous
"""Causal self-attention (B=2, N=2048, E=1024, H=16, HD=64) on 8 trn2 NeuronCores.

Sharding: (batch, head-group) — core c handles batch c//4 and heads
4*(c%4) .. 4*(c%4)+3.  Each core computes its heads' QKV projections,
causal attention, and a partial out-projection over its 256 feature rows
of Wout; the host sums the 4 partials per batch and adds all biases that
are affine in the output (bout and the v-bias term, which is constant
because softmax rows sum to 1).

v6 design (trace-driven):
  - ALL matmul operands are bf16 (fp32 psum accumulate): fp32 K=64
    row-tiled score matmuls ran at half rate; bf16 restores 1 col/cycle
    and FWL halves weight loads.  Input/output DMA bytes halve.
  - attention in q-chunks of 512 over 4 passes; per (k-tile, pair)
    sub-step both heads' S^T scores go to one [128,1024] psum tile with
    each head's 512-col block exactly one psum bank (no bank sharing:
    a shared-bank group ending in a K=64 matmul crashes the HW), pool
    bufs=2 so sub-steps double-buffer.  One exp ACT per sub-step over
    exact [128,2,w] segments.
  - causal mask on the PE: diagonal 128-blocks get an accumulating
    eye.T @ tmask matmul (K=128 group closer) before exp.
  - PV per pair accumulates into a [65,1024] psum tile (head blocks =
    own banks, K=128 groups); normalization is DEFERRED into the next
    pass as paced fillers: psum row 64 -> SBUF copy -> fast reciprocal
    (rcp straight from PSUM reads wrong data on HW) -> gpsimd
    partition broadcast -> per-head multiply into bf16 saT.
  - projections stream as paced PE filler; interior qk chunks (1,2) are
    produced two-at-a-time per stationary weight load (halves LDW);
    out-projection of chunk c runs in pass c+1 after c's norm; output
    DMA'd as bf16 (host sums partials in fp32).
  - HAM: QC=256 variant measured 172us clock-gated (small matmuls drop
    PE duty below the activity threshold) — keep matmuls >=256 cols.
"""

import numpy as np

import concourse.bass as bass
import concourse.tile as tile
from concourse import bacc, mybir
from concourse import bass_utils

B, N, E, H = 2, 2048, 1024, 16
HD = 64
NCORES = 8
NE = E // 128      # 8 e-chunks
NK = N // 128      # 16 k-tiles
NQ = N // 512      # 4 xt/attention chunks of 512
F32 = mybir.dt.float32
BF16 = mybir.dt.bfloat16

_CACHE = {}


def _build_body(nc, tc, pools, dram, rep):
    xt_d, wqk_d, wv_d, wout_d, bqk_d, eye_d, tmask_d, ones_d, out_d = dram
    (pconst, pqk, pvext, psa, pesb, psmall, pbc, pout, psps, ppv) = pools
    Exp = mybir.ActivationFunctionType.Exp

    # ---- input DMA: two HWDGE rings, big transfers, first-use order ------
    eye_sb = pconst.tile([128, 128], BF16, tag="eye", name="eye")
    nc.scalar.dma_start(eye_sb[:], eye_d[:])
    tmask_sb = pconst.tile([128, 128], BF16, tag="tmask", name="tmask")
    nc.scalar.dma_start(tmask_sb[:], tmask_d[:])
    ones_sb = pconst.tile([128, 4], BF16, tag="ones", name="ones")
    nc.scalar.dma_start(ones_sb[:], ones_d[:])
    bias_sb = {}
    for p in range(2):
        for i, nm in enumerate(("bq", "bk")):
            t = pconst.tile([128, 1], F32, tag=f"{nm}{p}", name=f"{nm}{p}")
            nc.scalar.dma_start(t[:], bqk_d[p, i])
            bias_sb[(p, nm)] = t
    wqk_sb = pconst.tile([128, NE * 512], BF16, tag="wqk", name="wqk")
    nc.scalar.dma_start(wqk_sb[:], wqk_d[:])
    xt = []
    for nq in range(NQ):
        t = pconst.tile([128, NE * 512], BF16, tag=f"xt{nq}", name=f"xt{nq}")
        nc.sync.dma_start(t[:], xt_d[nq])
        xt.append(t)
    wv_sb = pconst.tile([128, NE * 256], BF16, tag="wv", name="wv")
    nc.scalar.dma_start(wv_sb[:], wv_d[:])
    wout_sb = pconst.tile([128, 2 * E], BF16, tag="wout", name="wout")
    nc.scalar.dma_start(wout_sb[:], wout_d[:])

    # ---- PE warmup while the DMAs stream: release the HAM clock gate -----
    wps = psps.tile([128, 128], F32, tag="sps", name="warmps")
    for i in range(28):
        nc.tensor.matmul(wps[:], eye_sb[:], eye_sb[:],
                         start=(i == 0), stop=(i == 27))
    wsb = psmall.tile([128, 128], F32, tag="warm", name="warmsb")
    nc.vector.tensor_copy(wsb[:], wps[:])

    # ---- persistent SBUF tensors ----------------------------------------
    qT, kT, saT = {}, {}, {}
    for p in range(2):
        qT[p] = pqk.tile([128, N], BF16, tag=f"qT{p}", name=f"qT{p}")
        kT[p] = pqk.tile([128, N], BF16, tag=f"kT{p}", name=f"kT{p}")
        saT[p] = psa.tile([128, N], BF16, tag=f"saT{p}", name=f"saT{p}")
    v_ext = []
    for nk in range(NK):
        vt = pvext.tile([128, 4 * 65], BF16, tag=f"vext{nk}", name=f"vext{nk}")
        # ones columns via DVE copy from a DMA'd tile (memset tiles are
        # HW-hazardous as PE operands; DMA+copy is proven)
        nc.vector.tensor_copy(
            vt[:].rearrange("p (h d) -> p h d", h=4)[:, :, 64:65],
            ones_sb[:].rearrange("p (h d) -> p h d", d=1))
        v_ext.append(vt)

    def emit_qkproj(p, which, nqs):
        """Project q or k for pair p for one or two 512-col chunks,
        sharing each stationary weight load across the chunks."""
        off = 0 if which == "k" else 256
        bias = bias_sb[(p, "bq" if which == "q" else "bk")]
        dst = qT[p] if which == "q" else kT[p]
        ps = psps.tile([128, 512 * len(nqs)], F32, tag="sps", name="projps")
        for e in range(NE):
            for j, nq in enumerate(nqs):
                nc.tensor.matmul(
                    ps[:, j * 512:(j + 1) * 512],
                    wqk_sb[:, e * 512 + off + p * 128:e * 512 + off + p * 128 + 128],
                    xt[nq][:, e * 512:(e + 1) * 512],
                    start=(e == 0), stop=(e == NE - 1))
        for j, nq in enumerate(nqs):
            nc.vector.tensor_scalar_add(
                dst[:, nq * 512:(nq + 1) * 512],
                ps[:, j * 512:(j + 1) * 512], bias[:])

    def emit_vproj_group(nk):
        if nk >= NK:
            return
        ps = psps.tile([128, 256], F32, tag="sps", name="vps")
        for e in range(NE):
            nc.tensor.matmul(
                ps[:], xt[nk // 4][:, e * 512 + (nk % 4) * 128:e * 512 + (nk % 4) * 128 + 128],
                wv_sb[:, e * 256:(e + 1) * 256],
                start=(e == 0), stop=(e == NE - 1))
        nc.vector.tensor_copy(
            v_ext[nk][:].rearrange("p (h d) -> p h d", h=4)[:, :, 0:64],
            ps[:].rearrange("p (h d) -> p h d", h=4))

    def emit_outproj(nk):
        ot = pout.tile([128, E], BF16, tag="outsb", name="outsb")
        for oc in range(2):
            ps = psps.tile([128, 512], F32, tag="sps", name="ops")
            for p in range(2):
                nc.tensor.matmul(
                    ps[:], saT[p][:, nk * 128:(nk + 1) * 128],
                    wout_sb[:, p * E + oc * 512:p * E + oc * 512 + 512],
                    start=(p == 0), stop=(p == 1))
            nc.vector.tensor_copy(ot[:, oc * 512:(oc + 1) * 512], ps[:])
        nc.sync.dma_start(out_d[nk], ot[:])

    def emit_norm_chain(qi, pvs):
        """Normalize pass qi's pv accumulators into saT.  Returns
        (drain, rest): `drain` copies both pv psum tiles to bf16 SBUF
        (frees the psum banks in ~1.3us so pass qi+1's PV never stalls);
        `rest` runs the reciprocal / broadcast / multiply chain from
        SBUF off the critical path."""
        st = {}

        def c_drain():
            for p in range(2):
                pvsb = psmall.tile([65, 1024], BF16, tag="pvsb",
                                   name=f"pvsb{qi}_{p}")
                nc.vector.tensor_copy(pvsb[:], pvs[p][:])
                st[("pvsb", p)] = pvsb

        rest = []
        for p in range(2):
            def c_rcp(p=p):
                # rcp_approx_fast needs fp32 in/out (and a PSUM source
                # reads wrong data on HW): convert the bf16 den row
                den32 = psmall.tile([1, 1024], F32, tag="dcp",
                                    name=f"dcp{qi}_{p}")
                nc.vector.tensor_copy(den32[:], st[("pvsb", p)][64:65, :])
                den = psmall.tile([1, 1024], F32, tag="den",
                                  name=f"den{qi}_{p}")
                nc.vector.reciprocal_approx_fast(den[:], den32[:])
                st[("den", p)] = den

            def c_bcast(p=p):
                bc = pbc.tile([64, 1024], F32, tag="bc", name=f"bc{qi}_{p}")
                nc.gpsimd.partition_broadcast(bc[:], st[("den", p)][:])
                st[("bc", p)] = bc

            def c_muls(p=p):
                for hh in range(2):
                    nc.vector.tensor_mul(
                        saT[p][hh * 64:hh * 64 + 64, qi * 512:(qi + 1) * 512],
                        st[("pvsb", p)][0:64, hh * 512:(hh + 1) * 512],
                        st[("bc", p)][:, hh * 512:(hh + 1) * 512])
            rest += [c_rcp, c_bcast, c_muls]
        return [c_drain], rest

    def attn_pass(qi, fillers):
        """Attention for q-chunk qi (512 wide), both pairs, four heads.
        Per (k-tile, pair) sub-step: two K=64 row-tiled score MMs (one
        per head, each into its OWN psum bank), causal mask accumulate
        on diagonal blocks, one exact segmented exp ACT; PV lags one
        k-tile.  Returns {p: pv tile} for deferred normalization."""
        nsteps = 4 * qi + 4
        pvs = {p: ppv.tile([65, 1024], F32, tag="pv", name=f"pv{qi}_{p}")
               for p in range(2)}
        hi = qi * 512 + 512
        pending = []
        nfill = 0
        nsub = nsteps * 2

        def emit_pv(kj, lo, p, esb):
            col0 = lo - qi * 512
            for hh in range(2):
                hloc = 2 * p + hh
                nc.tensor.matmul(
                    pvs[p][:, hh * 512 + col0:hh * 512 + 512],
                    v_ext[kj][:, hloc * 65:hloc * 65 + 65],
                    esb[:, hh * 512 + col0:hh * 512 + hi - lo + col0],
                    start=(kj == 0), stop=(kj == 4 * qi + 3))

        for kj in range(nsteps):
            q0 = 128 * kj
            lo = max(qi * 512, q0)
            w = hi - lo
            col0 = lo - qi * 512
            diag = (lo == q0)
            for p in range(2):
                sps = psps.tile([128, 1024], F32, tag="sps", name="sps")
                for hh in range(2):
                    rb = hh * 64
                    nc.tensor.matmul(
                        sps[:, hh * 512 + col0:hh * 512 + col0 + w],
                        kT[p][rb:rb + 64, q0:q0 + 128],
                        qT[p][rb:rb + 64, lo:hi],
                        start=True, stop=not diag)
                    if diag:
                        nc.tensor.matmul(
                            sps[:, hh * 512 + col0:hh * 512 + col0 + 128],
                            eye_sb[:], tmask_sb[:],
                            start=False, stop=True)
                esb = pesb.tile([128, 1024], BF16, tag="esb", name="esb")
                src = sps[:].rearrange("p (h c) -> p h c", h=2)[:, :, col0:col0 + w]
                dst = esb[:].rearrange("p (h c) -> p h c", h=2)[:, :, col0:col0 + w]
                nc.scalar.activation(dst, src, Exp)
                target = ((2 * kj + p + 1) * len(fillers)) // nsub
                while nfill < target:
                    fillers[nfill]()
                    nfill += 1
                pending.append((kj, lo, p, esb))
                if len(pending) > 2:
                    emit_pv(*pending.pop(0))
        while pending:
            emit_pv(*pending.pop(0))
        while nfill < len(fillers):
            fillers[nfill]()
            nfill += 1
        return pvs

    # ---- schedule -------------------------------------------------------
    # Prefix: qk chunk 0 and v_ext[0..3] (all pass 0 needs).  Pass qi
    # fillers: norm of pass qi-1, then projections for later passes
    # (chunks 1+2 produced two-at-a-time per weight load), then the
    # out-projection of chunk qi-1 (its norm ran earlier this pass).
    for p in range(2):
        emit_qkproj(p, "k", [0])
        emit_qkproj(p, "q", [0])
    for nk in range(4):
        emit_vproj_group(nk)

    pv_prev = None
    for qi in range(NQ):
        drain, rest = ([], [])
        if pv_prev is not None:
            drain, rest = emit_norm_chain(qi - 1, pv_prev)
        proj = []
        if qi == 0:
            for p in range(2):
                proj.append(lambda p=p: emit_qkproj(p, "k", [1, 2]))
                proj.append(lambda p=p: emit_qkproj(p, "q", [1, 2]))
        elif qi == 2:
            for p in range(2):
                proj.append(lambda p=p: emit_qkproj(p, "k", [3]))
                proj.append(lambda p=p: emit_qkproj(p, "q", [3]))
        for nk in range(4 * qi + 4, 4 * qi + 8):
            if nk < NK:
                proj.append(lambda nk=nk: emit_vproj_group(nk))
        # drain first (frees pv psum), then the norm chain interleaved
        # with PE-feeding projections, then the out-projections (which
        # read saT written by the chain's final muls)
        fillers = list(drain)
        for i in range(max(len(rest), len(proj))):
            if i < len(rest):
                fillers.append(rest[i])
            if i < len(proj):
                fillers.append(proj[i])
        if qi >= 1:
            for nk in range(4 * (qi - 1), 4 * qi):
                fillers.append(lambda nk=nk: emit_outproj(nk))
        pv_prev = attn_pass(qi, fillers)

    drain, rest = emit_norm_chain(NQ - 1, pv_prev)
    for fn in drain + rest:
        fn()
    for nk in range(12, NK):
        emit_outproj(nk)


def build_nc(reps=1, loop=None):
    nc = bacc.Bacc("TRN2", target_bir_lowering=False, debug=False,
                   enable_asserts=True, num_devices=NCORES)
    xt_d = nc.dram_tensor("xt", [NQ, 128, NE * 512], BF16,
                          kind="ExternalInput").ap()
    wqk_d = nc.dram_tensor("wqk", [128, NE * 512], BF16,
                           kind="ExternalInput").ap()
    wv_d = nc.dram_tensor("wv", [128, NE * 256], BF16,
                          kind="ExternalInput").ap()
    wout_d = nc.dram_tensor("wout", [128, 2 * E], BF16,
                            kind="ExternalInput").ap()
    bqk_d = nc.dram_tensor("bqk", [2, 2, 128, 1], F32,
                           kind="ExternalInput").ap()
    eye_d = nc.dram_tensor("eye", [128, 128], BF16, kind="ExternalInput").ap()
    tmask_d = nc.dram_tensor("tmask", [128, 128], BF16,
                             kind="ExternalInput").ap()
    ones_d = nc.dram_tensor("ones", [128, 4], BF16,
                            kind="ExternalInput").ap()
    out_d = nc.dram_tensor("out", [NK, 128, E], BF16,
                           kind="ExternalOutput").ap()
    dram = (xt_d, wqk_d, wv_d, wout_d, bqk_d, eye_d, tmask_d, ones_d, out_d)

    with tile.TileContext(nc) as tc:
        from contextlib import ExitStack
        with ExitStack() as ctx:
            pconst = ctx.enter_context(tc.tile_pool(name="const", bufs=1))
            pqk = ctx.enter_context(tc.tile_pool(name="qk", bufs=1))
            pvext = ctx.enter_context(tc.tile_pool(name="vext", bufs=1))
            psa = ctx.enter_context(tc.tile_pool(name="sa", bufs=1))
            pesb = ctx.enter_context(tc.tile_pool(name="esb", bufs=4))
            psmall = ctx.enter_context(tc.tile_pool(name="small", bufs=2))
            pbc = ctx.enter_context(tc.tile_pool(name="bc", bufs=2))
            pout = ctx.enter_context(tc.tile_pool(name="outsb", bufs=2))
            psps = ctx.enter_context(
                tc.tile_pool(name="sps", bufs=2, space="PSUM"))
            ppv = ctx.enter_context(
                tc.tile_pool(name="pv", bufs=2, space="PSUM"))
            pools = (pconst, pqk, pvext, psa, pesb, psmall, pbc, pout,
                     psps, ppv)
            if loop is not None:
                with tc.For_i(0, loop, 1,
                              hint_engines=(mybir.EngineType.PE,
                                            mybir.EngineType.Activation,
                                            mybir.EngineType.DVE,
                                            mybir.EngineType.SP)):
                    _build_body(nc, tc, pools, dram, 0)
            else:
                for r in range(reps):
                    _build_body(nc, tc, pools, dram, r)
    nc.compile()
    return nc


def make_in_maps(x, Wqkv, bqkv, Wout):
    """Per-core input dicts. Shapes per reference: x[B,N,E], Wqkv[H,E,3HD],
    bqkv[H,3HD], Wout[E,E].  Split: cols 0:64=k, 64:128=q, 128:192=v."""
    import ml_dtypes
    bf16 = ml_dtypes.bfloat16
    Wk = Wqkv[:, :, 0:HD]
    Wq = Wqkv[:, :, HD:2 * HD] * (1.0 / np.sqrt(HD))
    Wv = Wqkv[:, :, 2 * HD:3 * HD]
    bk = bqkv[:, 0:HD]
    bq = bqkv[:, HD:2 * HD] * (1.0 / np.sqrt(HD))

    eye = np.eye(128, dtype=np.float32)
    # S^T tile rows are k, cols are q: keep q >= k -> add -1e9 where q < k
    tmask = np.tril(np.full((128, 128), -1e9, dtype=np.float32), k=-1)

    in_maps = []
    for c in range(NCORES):
        b, hg = divmod(c, 4)
        hs = slice(4 * hg, 4 * hg + 4)

        # xt[nq, row, e*512 + col] = x[b, nq*512+col, e*128+row]
        xT = np.ascontiguousarray(x[b].T)          # [E, N]
        xt = (xT.reshape(NE, 128, NQ, 512)
                .transpose(2, 1, 0, 3)
                .reshape(NQ, 128, NE * 512))

        # wqk[row, e*512 + which*256 + p*128 + hh*64 + d]
        #   = W_which[4hg + 2p + hh, e*128 + row, d]
        wqk = np.empty((128, NE, 2, 2, 2, HD), dtype=np.float32)
        for which, W in ((0, Wk), (1, Wq)):
            w4 = np.asarray(W[hs]).reshape(2, 2, NE, 128, HD)
            wqk[:, :, which] = w4.transpose(3, 2, 0, 1, 4)
        wqk = wqk.reshape(128, NE * 512)

        # wv[row, e*256 + hloc*64 + d] = Wv[4hg + hloc, e*128 + row, d]
        wv = (np.asarray(Wv[hs]).reshape(4, NE, 128, HD)
                .transpose(2, 1, 0, 3).reshape(128, NE * 256))

        # wout[row, p*E + c] = Wout[4hg*HD + p*128 + row, c]
        wout = (np.asarray(Wout[4 * hg * HD:(4 * hg + 4) * HD])
                .reshape(2, 128, E).transpose(1, 0, 2).reshape(128, 2 * E))

        bqk = np.stack([
            np.stack([bq[4 * hg + 2 * p:4 * hg + 2 * p + 2].reshape(128),
                      bk[4 * hg + 2 * p:4 * hg + 2 * p + 2].reshape(128)])
            for p in range(2)]).reshape(2, 2, 128, 1)
        in_maps.append({
            "xt": np.ascontiguousarray(xt).astype(bf16),
            "wqk": np.ascontiguousarray(wqk).astype(bf16),
            "wv": np.ascontiguousarray(wv).astype(bf16),
            "wout": np.ascontiguousarray(wout).astype(bf16),
            "bqk": bqk.astype(np.float32),
            "eye": eye.astype(bf16),
            "tmask": tmask.astype(bf16),
            "ones": np.ones((128, 4), dtype=np.float32).astype(bf16),
        })
    return in_maps


def combine(results, bqkv, Wout, bout):
    bv = bqkv[:, 2 * HD:3 * HD].reshape(E)          # concat over heads
    const_row = bv @ Wout + bout                     # [E]
    out = np.zeros((B, N, E), dtype=np.float32)
    for c in range(NCORES):
        b = c // 4
        out[b] += results[c]["out"].reshape(N, E).astype(np.float32)
    out += const_row[None, None, :].astype(np.float32)
    return out


def kernel(x, Wqkv, bqkv, Wout, bout):
    x = np.asarray(x, dtype=np.float32)
    Wqkv = np.asarray(Wqkv, dtype=np.float32)
    bqkv = np.asarray(bqkv, dtype=np.float32)
    Wout = np.asarray(Wout, dtype=np.float32)
    bout = np.asarray(bout, dtype=np.float32)

    if "nc" not in _CACHE:
        _CACHE["nc"] = build_nc(reps=1)
    nc = _CACHE["nc"]
    in_maps = make_in_maps(x, Wqkv, bqkv, Wout)
    res = bass_utils.run_bass_kernel_spmd(
        nc, in_maps, core_ids=list(range(NCORES)), trace=False)
    return combine(res.results, bqkv, Wout, bout)
